# revision 10
# baseline (speedup 1.0000x reference)
"""MoE + LoRA expert FFN kernel for 8 Trainium2 NeuronCores.

Strategy (expert-parallel, host dispatch/combine):
  - E=8 experts, one expert per core. The host groups tokens by expert
    (a token appears once per distinct selected expert; duplicate
    selections collapse with summed routing weight), pads each group to
    a uniform capacity C (multiple of 128), and ships per-core inputs:
        xT  [H, C]  bf16   tokens routed to this core's expert, transposed
        wg  [H, I]  bf16   gate_proj + 2*gate_A@gate_B   (LoRA folded)
        wu  [H, I]  bf16   up_proj   + 2*up_A@up_B
        wd  [I, H]  bf16   down_proj + 2*down_A@down_B
    and receives yT [H, C] fp32 = (silu(x@wg) * (x@wu)) @ wd, transposed.
  - Everything on device stays feature-major (features on partitions,
    tokens on the moving free dim) so no transposes are needed.
  - The host scales each token's expert output by its routing weight and
    scatters back into the [T, H] result.

LoRA folding is exact algebra: x@W + s*(x@A)@B == x@(W + s*A@B).
"""

import numpy as np
import ml_dtypes

E, H, I, R, TOPK = 8, 1024, 2816, 8, 2
SCALING = 2.0
NCORES = 8
KP = 128          # partition / contraction tile
NTOK = 512        # moving-dim (token) tile
BF16 = ml_dtypes.bfloat16

_cache = {}


def _setup_paths():
    import sys
    for p in ("/opt/trn_rl_repo", "/root/.axon_site"):
        if p not in sys.path:
            sys.path.insert(0, p)


def _split_multi_waits(nc):
    """The walrus in this container accepts at most 1 sem wait per
    instruction (2 on EventSemaphore); Tile emits more. Rewrite each block,
    moving excess waits onto preceding single-wait NoOps on the same
    engine (engines execute in order, so semantics are preserved)."""
    _setup_paths()
    from bass_rust import SyncInfo
    from concourse import mybir

    ctr = [0]
    for f in nc.m.functions:
        for bb in f.blocks:
            insts = bb.instructions
            new = []
            changed = False
            for inst in insts:
                si = inst.sync_info
                waits = list(si.on_wait or []) if si is not None else []
                cap = 2 if isinstance(inst, mybir.InstEventSemaphore) else 1
                if len(waits) > cap:
                    changed = True
                    for w in waits[:-cap]:
                        nop = mybir.InstNoOp(
                            name=f"SW-{ctr[0]}", ins=[], outs=[])
                        ctr[0] += 1
                        nop.engine = inst.engine
                        nop.sync_info = SyncInfo(on_wait=[w], on_update=[])
                        new.append(nop)
                    inst.sync_info = SyncInfo(
                        on_wait=waits[-cap:],
                        on_update=list(si.on_update or []))
                new.append(inst)
            if changed:
                bb.instructions = new


def _token_tiles(C):
    tiles = []
    t0 = 0
    while t0 < C:
        tw = min(NTOK, C - t0)
        tiles.append((t0, tw))
        t0 += tw
    return tiles


def _build(C):
    """Build the per-core Bass program for token capacity C (mult of 128)."""
    _setup_paths()
    import concourse.bass as bass
    import concourse.tile as tile
    from concourse import mybir

    bf16 = mybir.dt.bfloat16
    f32 = mybir.dt.float32
    KH = H // KP            # 8 contraction chunks over H
    KI = I // KP            # 22 chunks over I

    nc = bass.Bass("TRN2", target_bir_lowering=False, debug=False,
                   num_devices=NCORES)
    xT = nc.declare_dram_parameter("xT", [H, C], bf16, isOutput=False)
    wg = nc.declare_dram_parameter("wg", [H, I], bf16, isOutput=False)
    wu = nc.declare_dram_parameter("wu", [H, I], bf16, isOutput=False)
    wd = nc.declare_dram_parameter("wd", [I, H], bf16, isOutput=False)
    yT = nc.declare_dram_parameter("yT", [H, C], f32, isOutput=True)

    ttiles = _token_tiles(C)

    IG = 2                  # i-tiles per wg/wu DMA column group
    NQ = -(-KI // IG)       # 11 column groups

    with tile.TileContext(nc) as tc:
        with tc.tile_pool(name="xh", bufs=1) as xh, \
             tc.tile_pool(name="wd", bufs=6) as wdp, \
             tc.tile_pool(name="yout", bufs=4) as yp:
            # x split per (k, token-tile) so the first matmuls start early;
            # loads issued from the (idle) vector engine's queues
            x_t = {}
            for k in range(KH):
                for ti, (t0, tw) in enumerate(ttiles):
                    t = xh.tile([KP, tw], bf16, tag=f"x{k}_{ti}",
                                name=f"x{k}_{ti}")
                    nc.scalar.dma_start(
                        out=t, in_=xT[k * KP:(k + 1) * KP, t0:t0 + tw])
                    x_t[(k, ti)] = t
            h_t = [xh.tile([KP, C], bf16, tag=f"h{i}", name=f"h{i}")
                   for i in range(KI)]

            # ---- phase B: h = silu(x@wg) * (x@wu), feature-major [I, C]
            with tc.tile_pool(name="wgu", bufs=1) as wp, \
                 tc.tile_pool(name="warm", bufs=1, space="PSUM") as warm, \
                 tc.tile_pool(name="psB", bufs=3, space="PSUM") as psB, \
                 tc.tile_pool(name="actB", bufs=4) as actB:
                # ~4us of dummy matmuls so the PE HAM un-throttles to
                # 2.4 GHz while the first weight DMAs are in flight
                wsrc = actB.tile([KP, 256], bf16, tag="wsrc", name="wsrc")
                nc.vector.memset(wsrc, 0.0)
                wdst = warm.tile([KP, 256], f32, tag="wdst", name="wdst")
                for w in range(18):
                    nc.tensor.matmul(wdst, wsrc[:, :128], wsrc,
                                     start=(w == 0), stop=(w == 17))
                # column-grouped weight loads, ordered so group q arrives
                # before the i-tiles that need it
                wg_t, wu_t = {}, {}
                for q in range(NQ):
                    c0 = q * IG * KP
                    cw = min(IG * KP, I - c0)
                    for k in range(KH):
                        t = wp.tile([KP, cw], bf16, tag=f"wg{k}_{q}",
                                    name=f"wg{k}_{q}")
                        nc.sync.dma_start(
                            out=t, in_=wg[k * KP:(k + 1) * KP, c0:c0 + cw])
                        wg_t[(k, q)] = t
                        t = wp.tile([KP, cw], bf16, tag=f"wu{k}_{q}",
                                    name=f"wu{k}_{q}")
                        nc.scalar.dma_start(
                            out=t, in_=wu[k * KP:(k + 1) * KP, c0:c0 + cw])
                        wu_t[(k, q)] = t

                for i in range(KI):
                    q, r = divmod(i, IG)
                    isl = slice(r * KP, (r + 1) * KP)
                    for ti, (t0, tw) in enumerate(ttiles):
                        g_ps = psB.tile([KP, tw], f32, tag="g",
                                        name=f"g{i}_{t0}")
                        u_ps = psB.tile([KP, tw], f32, tag="u",
                                        name=f"u{i}_{t0}")
                        for k in range(KH):
                            nc.tensor.matmul(
                                g_ps, wg_t[(k, q)][:, isl], x_t[(k, ti)],
                                start=(k == 0), stop=(k == KH - 1))
                        for k in range(KH):
                            nc.tensor.matmul(
                                u_ps, wu_t[(k, q)][:, isl], x_t[(k, ti)],
                                start=(k == 0), stop=(k == KH - 1))
                        sg = actB.tile([KP, tw], f32, tag="sg",
                                       name=f"sg{i}_{t0}")
                        nc.scalar.activation(
                            sg, g_ps, mybir.ActivationFunctionType.Silu)
                        nc.vector.tensor_mul(
                            h_t[i][:, t0:t0 + tw], sg, u_ps)

            # ---- phase D: yT = h @ wd, output [H, C]
            # wd is streamed (re-loaded per token tile) from a small pool
            # allocated up-front, so its DMAs overlap phase B; each load is
            # split in two so it lands via two queues.
            with tc.tile_pool(name="psD", bufs=1, space="PSUM") as psD:
                HH = H // KP    # 8 output row blocks
                for ti, (t0, tw) in enumerate(ttiles):
                    y_ps = [psD.tile([KP, tw], f32, tag=f"y{hh}",
                                     name=f"y{hh}_{t0}")
                            for hh in range(HH)]
                    for i in range(KI):
                        wdt = wdp.tile([KP, H], bf16, tag="wd",
                                       name=f"wd{i}_{ti}")
                        nc.sync.dma_start(
                            out=wdt[:, :H // 2],
                            in_=wd[i * KP:(i + 1) * KP, :H // 2])
                        nc.sync.dma_start(
                            out=wdt[:, H // 2:],
                            in_=wd[i * KP:(i + 1) * KP, H // 2:])
                        for hh in range(HH):
                            nc.tensor.matmul(
                                y_ps[hh],
                                wdt[:, hh * KP:(hh + 1) * KP],
                                h_t[i][:, t0:t0 + tw],
                                start=(i == 0), stop=(i == KI - 1))
                    for hh in range(HH):
                        yo = yp.tile([KP, tw], f32, tag="yo",
                                     name=f"yo{hh}_{t0}")
                        nc.vector.tensor_copy(yo, y_ps[hh])
                        half = tw // 2
                        nc.scalar.dma_start(
                            out=yT[hh * KP:(hh + 1) * KP, t0:t0 + half],
                            in_=yo[:, :half])
                        nc.sync.dma_start(
                            out=yT[hh * KP:(hh + 1) * KP,
                                   t0 + half:t0 + tw],
                            in_=yo[:, half:])
    _split_multi_waits(nc)
    return nc


def _prepare(inputs):
    """Host-side routing + weight folding. Returns (in_maps, idx, wts, C)."""
    hs = np.asarray(inputs["hidden_states"], dtype=np.float32)
    rw = np.asarray(inputs["routing_weights"], dtype=np.float32)
    se = np.asarray(inputs["selected_experts"]).astype(np.int64)
    T = hs.shape[0]

    combine = np.zeros((T, E), dtype=np.float32)
    for k in range(se.shape[1]):
        np.add.at(combine, (np.arange(T), se[:, k]), rw[:, k])

    idx = [np.nonzero(combine[:, e])[0] for e in range(E)]
    wts = [combine[idx[e], e] for e in range(E)]
    maxn = max((len(ix) for ix in idx), default=1)
    C = max(KP, -(-maxn // KP) * KP)

    gp = np.asarray(inputs["gate_proj"], dtype=np.float32)
    up = np.asarray(inputs["up_proj"], dtype=np.float32)
    dp = np.asarray(inputs["down_proj"], dtype=np.float32)
    gA = np.asarray(inputs["gate_A"], dtype=np.float32)
    gB = np.asarray(inputs["gate_B"], dtype=np.float32)
    uA = np.asarray(inputs["up_A"], dtype=np.float32)
    uB = np.asarray(inputs["up_B"], dtype=np.float32)
    dA = np.asarray(inputs["down_A"], dtype=np.float32)
    dB = np.asarray(inputs["down_B"], dtype=np.float32)

    in_maps = []
    for e in range(E):
        n = len(idx[e])
        xTe = np.zeros((H, C), dtype=BF16)
        if n:
            xTe[:, :n] = hs[idx[e]].T.astype(BF16)
        wge = (gp[e] + SCALING * (gA[e] @ gB[e])).astype(BF16)
        wue = (up[e] + SCALING * (uA[e] @ uB[e])).astype(BF16)
        wde = (dp[e] + SCALING * (dA[e] @ dB[e])).astype(BF16)
        in_maps.append({"xT": xTe, "wg": wge, "wu": wue, "wd": wde})
    return in_maps, idx, wts, C


def kernel(**inputs):
    _setup_paths()
    from concourse.bass_utils import run_bass_kernel_spmd

    in_maps, idx, wts, C = _prepare(inputs)

    nc = _cache.get(C)
    if nc is None:
        nc = _build(C)
        _cache[C] = nc

    res = run_bass_kernel_spmd(nc, in_maps, core_ids=list(range(NCORES)))

    # expose for external profiling harnesses (test.py)
    kernel._last = {"nc": nc, "in_maps": in_maps, "results": res}

    hs = np.asarray(inputs["hidden_states"], dtype=np.float32)
    T = hs.shape[0]
    out = np.zeros((T, H), dtype=np.float32)
    for e in range(E):
        n = len(idx[e])
        if not n:
            continue
        yTe = res.results[e]["yT"]          # [H, C] fp32
        out[idx[e]] += wts[e][:, None] * yTe[:, :n].T
    return out


# revision 11
# speedup vs baseline: 1.2685x; 1.2685x over previous
"""MoE + LoRA expert FFN kernel for 8 Trainium2 NeuronCores.

Strategy (expert-parallel, host dispatch/combine):
  - E=8 experts, one expert per core. The host groups tokens by expert
    (a token appears once per distinct selected expert; duplicate
    selections collapse with summed routing weight), pads each group to
    a uniform capacity C (multiple of 128), and ships per-core inputs:
        xT  [H, C]  bf16   tokens routed to this core's expert, transposed
        wg  [H, I]  bf16   gate_proj + 2*gate_A@gate_B   (LoRA folded)
        wu  [H, I]  bf16   up_proj   + 2*up_A@up_B
        wd  [I, H]  bf16   down_proj + 2*down_A@down_B
    and receives yT [H, C] fp32 = (silu(x@wg) * (x@wu)) @ wd, transposed.
  - Everything on device stays feature-major (features on partitions,
    tokens on the moving free dim) so no transposes are needed.
  - The host scales each token's expert output by its routing weight and
    scatters back into the [T, H] result.

LoRA folding is exact algebra: x@W + s*(x@A)@B == x@(W + s*A@B).
"""

import numpy as np
import ml_dtypes

E, H, I, R, TOPK = 8, 1024, 2816, 8, 2
SCALING = 2.0
NCORES = 8
KP = 128          # partition / contraction tile
NTOK = 512        # moving-dim (token) tile
BF16 = ml_dtypes.bfloat16

_cache = {}


def _setup_paths():
    import sys
    for p in ("/opt/trn_rl_repo", "/root/.axon_site"):
        if p not in sys.path:
            sys.path.insert(0, p)


def _split_multi_waits(nc):
    """The walrus in this container accepts at most 1 sem wait per
    instruction (2 on EventSemaphore); Tile emits more. Rewrite each block,
    moving excess waits onto preceding single-wait NoOps on the same
    engine (engines execute in order, so semantics are preserved)."""
    _setup_paths()
    from bass_rust import SyncInfo
    from concourse import mybir

    ctr = [0]
    for f in nc.m.functions:
        for bb in f.blocks:
            insts = bb.instructions
            new = []
            changed = False
            for inst in insts:
                si = inst.sync_info
                waits = list(si.on_wait or []) if si is not None else []
                cap = 2 if isinstance(inst, mybir.InstEventSemaphore) else 1
                if len(waits) > cap:
                    changed = True
                    for w in waits[:-cap]:
                        nop = mybir.InstNoOp(
                            name=f"SW-{ctr[0]}", ins=[], outs=[])
                        ctr[0] += 1
                        nop.engine = inst.engine
                        nop.sync_info = SyncInfo(on_wait=[w], on_update=[])
                        new.append(nop)
                    inst.sync_info = SyncInfo(
                        on_wait=waits[-cap:],
                        on_update=list(si.on_update or []))
                new.append(inst)
            if changed:
                bb.instructions = new


def _token_tiles(C):
    tiles = []
    t0 = 0
    while t0 < C:
        tw = min(NTOK, C - t0)
        tiles.append((t0, tw))
        t0 += tw
    return tiles


def _build(C):
    """Build the per-core Bass program for token capacity C (mult of 128)."""
    _setup_paths()
    import concourse.bass as bass
    import concourse.tile as tile
    from concourse import mybir

    bf16 = mybir.dt.bfloat16
    f32 = mybir.dt.float32
    KH = H // KP            # 8 contraction chunks over H
    KI = I // KP            # 22 chunks over I

    nc = bass.Bass("TRN2", target_bir_lowering=False, debug=False,
                   num_devices=NCORES)
    xT = nc.declare_dram_parameter("xT", [H, C], bf16, isOutput=False)
    wg = nc.declare_dram_parameter("wg", [H, I], bf16, isOutput=False)
    wu = nc.declare_dram_parameter("wu", [H, I], bf16, isOutput=False)
    wd = nc.declare_dram_parameter("wd", [I, H], bf16, isOutput=False)
    yT = nc.declare_dram_parameter("yT", [H, C], f32, isOutput=True)

    ttiles = _token_tiles(C)

    IG = 4                  # i-tiles per wg/wu DMA column group
    NQ = -(-KI // IG)       # 11 column groups

    with tile.TileContext(nc) as tc:
        with tc.tile_pool(name="xh", bufs=1) as xh, \
             tc.tile_pool(name="wd", bufs=6) as wdp, \
             tc.tile_pool(name="yout", bufs=4) as yp:
            # x loads issued from ACT (strictly before any ACT compute)
            x_t = []
            for k in range(KH):
                t = xh.tile([KP, C], bf16, tag=f"x{k}", name=f"x{k}")
                nc.scalar.dma_start(
                    out=t, in_=xT[k * KP:(k + 1) * KP, :])
                x_t.append(t)
            h_t = [xh.tile([KP, C], bf16, tag=f"h{i}", name=f"h{i}")
                   for i in range(KI)]

            # ---- phase B: h = silu(x@wg) * (x@wu), feature-major [I, C]
            with tc.tile_pool(name="wgu", bufs=1) as wp, \
                 tc.tile_pool(name="warm", bufs=1, space="PSUM") as warm, \
                 tc.tile_pool(name="psB", bufs=3, space="PSUM") as psB, \
                 tc.tile_pool(name="actB", bufs=4) as actB:
                # ~4us of dummy matmuls so the PE HAM un-throttles to
                # 2.4 GHz while the first weight DMAs are in flight
                wsrc = actB.tile([KP, 256], bf16, tag="wsrc", name="wsrc")
                nc.vector.memset(wsrc, 0.0)
                wdst = warm.tile([KP, 256], f32, tag="wdst", name="wdst")
                for w in range(18):
                    nc.tensor.matmul(wdst, wsrc[:, :128], wsrc,
                                     start=(w == 0), stop=(w == 17))
                # column-grouped weight loads, ordered so group q arrives
                # before the i-tiles that need it
                wg_t, wu_t = {}, {}
                for q in range(NQ):
                    c0 = q * IG * KP
                    cw = min(IG * KP, I - c0)
                    for k in range(KH):
                        t = wp.tile([KP, cw], bf16, tag=f"wg{k}_{q}",
                                    name=f"wg{k}_{q}")
                        nc.sync.dma_start(
                            out=t, in_=wg[k * KP:(k + 1) * KP, c0:c0 + cw])
                        wg_t[(k, q)] = t
                        t = wp.tile([KP, cw], bf16, tag=f"wu{k}_{q}",
                                    name=f"wu{k}_{q}")
                        nc.sync.dma_start(
                            out=t, in_=wu[k * KP:(k + 1) * KP, c0:c0 + cw])
                        wu_t[(k, q)] = t

                for i in range(KI):
                    q, r = divmod(i, IG)
                    isl = slice(r * KP, (r + 1) * KP)
                    for ti, (t0, tw) in enumerate(ttiles):
                        g_ps = psB.tile([KP, tw], f32, tag="g",
                                        name=f"g{i}_{t0}")
                        u_ps = psB.tile([KP, tw], f32, tag="u",
                                        name=f"u{i}_{t0}")
                        for k in range(KH):
                            nc.tensor.matmul(
                                g_ps, wg_t[(k, q)][:, isl],
                                x_t[k][:, t0:t0 + tw],
                                start=(k == 0), stop=(k == KH - 1))
                        for k in range(KH):
                            nc.tensor.matmul(
                                u_ps, wu_t[(k, q)][:, isl],
                                x_t[k][:, t0:t0 + tw],
                                start=(k == 0), stop=(k == KH - 1))
                        sg = actB.tile([KP, tw], f32, tag="sg",
                                       name=f"sg{i}_{t0}")
                        nc.scalar.activation(
                            sg, g_ps, mybir.ActivationFunctionType.Silu)
                        nc.vector.tensor_mul(
                            h_t[i][:, t0:t0 + tw], sg, u_ps)

            # ---- phase D: yT = h @ wd, output [H, C]
            # wd is streamed (re-loaded per token tile) from a small pool
            # allocated up-front, so its DMAs overlap phase B; each load is
            # split in two so it lands via two queues.
            with tc.tile_pool(name="psD", bufs=1, space="PSUM") as psD:
                HH = H // KP    # 8 output row blocks
                for ti, (t0, tw) in enumerate(ttiles):
                    y_ps = [psD.tile([KP, tw], f32, tag=f"y{hh}",
                                     name=f"y{hh}_{t0}")
                            for hh in range(HH)]
                    for i in range(KI):
                        wdt = wdp.tile([KP, H], bf16, tag="wd",
                                       name=f"wd{i}_{ti}")
                        nc.sync.dma_start(
                            out=wdt, in_=wd[i * KP:(i + 1) * KP, :])
                        for hh in range(HH):
                            nc.tensor.matmul(
                                y_ps[hh],
                                wdt[:, hh * KP:(hh + 1) * KP],
                                h_t[i][:, t0:t0 + tw],
                                start=(i == 0), stop=(i == KI - 1))
                    for hh in range(HH):
                        yo = yp.tile([KP, tw], f32, tag="yo",
                                     name=f"yo{hh}_{t0}")
                        nc.vector.tensor_copy(yo, y_ps[hh])
                        nc.scalar.dma_start(
                            out=yT[hh * KP:(hh + 1) * KP, t0:t0 + tw],
                            in_=yo)
    _split_multi_waits(nc)
    return nc


def _prepare(inputs):
    """Host-side routing + weight folding. Returns (in_maps, idx, wts, C)."""
    hs = np.asarray(inputs["hidden_states"], dtype=np.float32)
    rw = np.asarray(inputs["routing_weights"], dtype=np.float32)
    se = np.asarray(inputs["selected_experts"]).astype(np.int64)
    T = hs.shape[0]

    combine = np.zeros((T, E), dtype=np.float32)
    for k in range(se.shape[1]):
        np.add.at(combine, (np.arange(T), se[:, k]), rw[:, k])

    idx = [np.nonzero(combine[:, e])[0] for e in range(E)]
    wts = [combine[idx[e], e] for e in range(E)]
    maxn = max((len(ix) for ix in idx), default=1)
    C = max(KP, -(-maxn // KP) * KP)

    gp = np.asarray(inputs["gate_proj"], dtype=np.float32)
    up = np.asarray(inputs["up_proj"], dtype=np.float32)
    dp = np.asarray(inputs["down_proj"], dtype=np.float32)
    gA = np.asarray(inputs["gate_A"], dtype=np.float32)
    gB = np.asarray(inputs["gate_B"], dtype=np.float32)
    uA = np.asarray(inputs["up_A"], dtype=np.float32)
    uB = np.asarray(inputs["up_B"], dtype=np.float32)
    dA = np.asarray(inputs["down_A"], dtype=np.float32)
    dB = np.asarray(inputs["down_B"], dtype=np.float32)

    in_maps = []
    for e in range(E):
        n = len(idx[e])
        xTe = np.zeros((H, C), dtype=BF16)
        if n:
            xTe[:, :n] = hs[idx[e]].T.astype(BF16)
        wge = (gp[e] + SCALING * (gA[e] @ gB[e])).astype(BF16)
        wue = (up[e] + SCALING * (uA[e] @ uB[e])).astype(BF16)
        wde = (dp[e] + SCALING * (dA[e] @ dB[e])).astype(BF16)
        in_maps.append({"xT": xTe, "wg": wge, "wu": wue, "wd": wde})
    return in_maps, idx, wts, C


def kernel(**inputs):
    _setup_paths()
    from concourse.bass_utils import run_bass_kernel_spmd

    in_maps, idx, wts, C = _prepare(inputs)

    nc = _cache.get(C)
    if nc is None:
        nc = _build(C)
        _cache[C] = nc

    res = run_bass_kernel_spmd(nc, in_maps, core_ids=list(range(NCORES)))

    # expose for external profiling harnesses (test.py)
    kernel._last = {"nc": nc, "in_maps": in_maps, "results": res}

    hs = np.asarray(inputs["hidden_states"], dtype=np.float32)
    T = hs.shape[0]
    out = np.zeros((T, H), dtype=np.float32)
    for e in range(E):
        n = len(idx[e])
        if not n:
            continue
        yTe = res.results[e]["yT"]          # [H, C] fp32
        out[idx[e]] += wts[e][:, None] * yTe[:, :n].T
    return out
